# revision 24
# baseline (speedup 1.0000x reference)
"""NT-Xent (SimCLR) loss on 8 Trainium2 NeuronCores.

Math (matches the jax reference):
    z = concat(proj_i, proj_j)            [N=4096, D=512]
    sim = cos(z_i, z_j) / T               T = 0.5
    loss = (1/N) sum_i [ logsumexp_{j!=i}(sim_i) - sim[i, label(i)] ]
    label(i) = (i + N/2) % N

Distribution: each core owns 512 rows of the 4096-row similarity matrix.
Each core receives the FULL feature-major z^T (bf16), column-rolled by its
row offset, so the kernel is a single static SPMD program:
  - columns [0,512)   = the core's own rows  (lhsT slice)
  - column  2048+m    = positive pair of local row m (static diagonal)
Row sums / logs are invariant to the column permutation, and the diagonal
self-term is exactly exp(2), subtracted as a constant.

Per-core pipeline (raw-Gram formulation — no normalize pass):
  DMA bf16 z^T chunks
  -> PE diag-block matmuls (G_bb per 128-block) -> DVE identity-mask +
     reduce extracts ||z_n||^2 partition-major  [128, 32]
  -> inv = exp(-0.5 ln nsq) on ACT (exp+ln share one table set; rsqrt/sqrt
     sets avoided), row-side variant 2*inv via bias=ln(2)
  -> PE transpose + reshape-DMA + broadcast-DMA => inv_bc [128, 4096]
  -> PE Gram slab G = z_loc^T z (bf16, raw)
  -> DVE fold: G * inv_col        (one pass, PSUM->SBUF)
  -> ACT exp(scale_AP * x), scale = 2*inv_row per partition, fused row-sum
     accumulation (accum_out)
  -> pos via identity-mask on the fold output; final ln(rowsum - e^2)
  -> [128,1] partial per core; host sums 8x128 partials.
"""

import sys

sys.path.insert(0, "/opt/trn_rl_repo")

from contextlib import ExitStack

import ml_dtypes
import numpy as np

import concourse.bacc as bacc
import concourse.bass as bass
import concourse.mybir as mybir
import concourse.tile as tile
from concourse import bass_utils
from concourse.masks import make_identity
from concourse.vector_clock import ScopedClock
from concourse.tile_rust import add_dep_helper


def _cheap_drain_and_barrier(self, tick_clock, wait_clock):
    """Kernel tail: skip the second all-engine barrier of the stock
    TileContext epilogue (drain -> barrier -> sem clear). The engine
    streams end right after the clears, so NEFF completion already
    orders them; saves several us of fixed tail."""
    drain_inst = self.nc.sync.drain()
    wait_clock.add_sem_waits(
        drain_inst.ins, ScopedClock({None: tick_clock.global_clock})
    )
    self.nc.all_engine_barrier()
    popped = self.nc._tile_sem_poison_stack.pop()
    assert popped is self._sem_poison
    self.nc.clear_and_free_semaphores(list(self.sems.allocated().values()))

P = 128                  # partitions
D = 512                  # feature dim
N = 4096                 # 2B rows
KT = D // P              # 4 contraction tiles
NCHUNK = 4               # column chunks (1MB DMAs)
CH = N // NCHUNK         # 1024
IBC = CH // P            # 8 diag blocks per chunk
NB = N // P              # 32 diag blocks
ROWS = N // 8            # 512 rows per core
MT = ROWS // P           # 4 m-tiles per core
GW = 1024                # gram/psum group width (2 PSUM banks)
NQ = N // GW             # 4 groups per m-tile row
POSQ = 2                 # group holding the positive-pair columns
INV_T = 2.0              # 1 / temperature
E2 = float(np.exp(2.0))
LN2 = float(np.log(2.0))

F32 = mybir.dt.float32
BF16 = mybir.dt.bfloat16
FP8 = mybir.dt.float8e4
AF = mybir.ActivationFunctionType
ALU = mybir.AluOpType


def _bc_mid(ap: bass.AP, count: int) -> bass.AP:
    """Broadcast a [P, X] AP over an inserted middle dim: [P, count, X]."""
    return bass.AP(tensor=ap.tensor, offset=ap.offset,
                   ap=[ap.ap[0], [0, count], ap.ap[1]])


def _build_body(ctx: ExitStack, tc: tile.TileContext, zt_ap: bass.AP, out_ap: bass.AP):
    nc = tc.nc
    ztp = zt_ap.rearrange("(k p) n -> p k n", p=P)

    const = ctx.enter_context(tc.tile_pool(name="const", bufs=1))
    big = ctx.enter_context(tc.tile_pool(name="big", bufs=1))
    work = ctx.enter_context(tc.tile_pool(name="work", bufs=3))
    small = ctx.enter_context(tc.tile_pool(name="small", bufs=1))
    ps = ctx.enter_context(tc.tile_pool(name="ps", bufs=1, space="PSUM"))

    ident = const.tile([P, P], F32)
    make_identity(nc, ident)
    neg_e2 = const.tile([P, 1], F32)
    nc.vector.memset(neg_e2, -E2)
    ones = const.tile([P, P], BF16)
    nc.vector.memset(ones, 1.0)

    zb = big.tile([P, KT, N], FP8)        # 16KB/part
    inv_bc = big.tile([P, N], F32)        # 16KB/part
    inv2_pm = small.tile([P, MT], F32)    # 2/||z||, local rows, partition-major
    rs = small.tile([P, MT, NQ], F32)
    pos4raw = small.tile([P, MT], F32)
    pos4 = small.tile([P, MT], F32)
    rsum4 = small.tile([P, MT], F32)
    lnoff4 = small.tile([P, MT], F32)
    res4 = small.tile([P, MT], F32)
    outsb = small.tile([P, 1], F32)

    # ---- phase 1: per chunk, DMA -> square (ACT/DVE split) -> norms^2 via
    # ones-matmul (partition reduction, output broadcast across partitions)
    # -> absrsqrt straight into inv_bc. No transposes, no DRAM bounces. ----
    for c in range(NCHUNK):
        cs = slice(c * CH, (c + 1) * CH)
        nc.sync.dma_start(out=zb[:, :, cs], in_=ztp[:, :, cs])
        zsq = work.tile([P, KT, CH], BF16, tag="zsq", bufs=2, name=f"zsq{c}")
        if c % 2 == 0:
            nc.scalar.activation(out=zsq, in_=zb[:, :, cs], func=AF.Square)
        else:
            nc.vector.tensor_mul(zsq, zb[:, :, cs], zb[:, :, cs])
        for half in range(2):
            s = c * CH + half * 512
            nsq_ps = ps.tile([P, 512], F32, tag="pre", bufs=2,
                             name=f"nsq{c}{half}")
            for k in range(KT):
                nc.tensor.matmul(
                    nsq_ps, lhsT=ones[:], rhs=zsq[:, k, half * 512 : (half + 1) * 512],
                    start=(k == 0), stop=(k == KT - 1),
                )
            nc.scalar.activation(out=inv_bc[:, s : s + 512], in_=nsq_ps,
                                 func=AF.Abs_reciprocal_sqrt)

    # ---- row-side scale 2/||z|| partition-major via tiny PE transposes of
    # the local 512 columns of inv_bc (values are partition-replicated) ----
    tps4 = ps.tile([P, MT, P], F32, tag="pre", bufs=2)
    for t in range(MT):
        nc.tensor.transpose(tps4[:, t, :], inv_bc[:, t * P : (t + 1) * P],
                            ident[:])
    nc.scalar.activation(out=inv2_pm, in_=tps4[:, :, 0], func=AF.Copy,
                         scale=2.0)

    # ---- phase 2: Gram slab + fold + exp row-sums + positives ----
    gi = 0
    for t in range(MT):
        ts_ = slice(t * P, (t + 1) * P)
        for q in range(NQ):
            gps = ps.tile([P, GW], F32, tag="gps", bufs=3, name=f"g{t}{q}")
            for k in range(KT):
                for j in range(GW // 512):
                    cols = slice(q * GW + j * 512, q * GW + (j + 1) * 512)
                    nc.tensor.matmul(
                        gps[:, j * 512 : (j + 1) * 512],
                        lhsT=zb[:, k, ts_], rhs=zb[:, k, cols],
                        start=(k == 0), stop=(k == KT - 1),
                    )
            gi += 1
            fold = work.tile([P, GW], F32, tag="fold", name=f"f{t}{q}")
            nc.vector.tensor_mul(fold, gps, inv_bc[:, q * GW : (q + 1) * GW])
            expo = work.tile([P, GW], BF16, tag="expo", bufs=2, name=f"e{t}{q}")
            nc.scalar.activation(
                out=expo, in_=fold, func=AF.Exp,
                scale=inv2_pm[:, t : t + 1],
                accum_out=rs[:, t, q : q + 1],
            )
            if q == POSQ:
                posscr = work.tile([P, P], F32, tag="posscr", name=f"p{t}")
                nc.vector.tensor_mul(posscr, fold[:, t * P : (t + 1) * P], ident)
                nc.vector.tensor_reduce(
                    out=pos4raw[:, t : t + 1], in_=posscr,
                    axis=mybir.AxisListType.X, op=ALU.add,
                )

    # ---- phase 3: per-row loss, partial reduction ----
    nc.vector.tensor_reduce(out=rsum4, in_=rs, axis=mybir.AxisListType.X,
                            op=ALU.add)
    nc.scalar.activation(out=lnoff4, in_=rsum4, func=AF.Ln, bias=neg_e2[:])
    nc.vector.tensor_mul(pos4, pos4raw, inv2_pm[:, 0:MT])
    nc.vector.tensor_sub(res4, lnoff4, pos4)
    nc.vector.tensor_reduce(out=outsb, in_=res4, axis=mybir.AxisListType.X,
                            op=ALU.add)
    nc.sync.dma_start(out=out_ap, in_=outsb)


_CACHE = {}


def _get_nc():
    if "nc" not in _CACHE:
        tile.TileContext._drain_and_barrier = _cheap_drain_and_barrier
        nc = bacc.Bacc("TRN2", target_bir_lowering=False, debug=False, num_devices=8)
        zt = nc.dram_tensor("zt", [D, N], FP8, kind="ExternalInput")
        out = nc.dram_tensor("out", [P, 1], F32, kind="ExternalOutput")
        with tile.TileContext(nc) as tc:
            with ExitStack() as ctx:
                _build_body(ctx, tc, zt.ap(), out.ap())
        nc.compile()
        _CACHE["nc"] = nc
    return _CACHE["nc"]


def _make_in_maps(proj_i: np.ndarray, proj_j: np.ndarray):
    z = np.concatenate(
        [np.asarray(proj_i, np.float32), np.asarray(proj_j, np.float32)], axis=0
    )
    zT = np.ascontiguousarray(z.T).astype(ml_dtypes.float8_e4m3fn)  # [D, N]
    return [
        {"zt": np.ascontiguousarray(np.roll(zT, -c * ROWS, axis=1))} for c in range(8)
    ]


def run(proj_i: np.ndarray, proj_j: np.ndarray, trace: bool = False):
    nc = _get_nc()
    in_maps = _make_in_maps(proj_i, proj_j)
    res = bass_utils.run_bass_kernel_spmd(
        nc, in_maps, core_ids=list(range(8)), trace=trace
    )
    loss = sum(float(r["out"].astype(np.float64).sum()) for r in res.results) / N
    return np.float32(loss), res


def kernel(proj_i: np.ndarray, proj_j: np.ndarray) -> np.ndarray:
    loss, _ = run(proj_i, proj_j)
    return np.asarray(loss, dtype=np.float32)


# revision 25
# speedup vs baseline: 1.0729x; 1.0729x over previous
"""NT-Xent (SimCLR) loss on 8 Trainium2 NeuronCores.

Math (matches the jax reference):
    z = concat(proj_i, proj_j)            [N=4096, D=512]
    sim = cos(z_i, z_j) / T               T = 0.5
    loss = (1/N) sum_i [ logsumexp_{j!=i}(sim_i) - sim[i, label(i)] ]
    label(i) = (i + N/2) % N

Distribution: each core owns 512 rows of the 4096-row similarity matrix.
Each core receives the FULL feature-major z^T (bf16), column-rolled by its
row offset, so the kernel is a single static SPMD program:
  - columns [0,512)   = the core's own rows  (lhsT slice)
  - column  2048+m    = positive pair of local row m (static diagonal)
Row sums / logs are invariant to the column permutation, and the diagonal
self-term is exactly exp(2), subtracted as a constant.

Per-core pipeline (raw-Gram formulation — no normalize pass):
  DMA bf16 z^T chunks
  -> PE diag-block matmuls (G_bb per 128-block) -> DVE identity-mask +
     reduce extracts ||z_n||^2 partition-major  [128, 32]
  -> inv = exp(-0.5 ln nsq) on ACT (exp+ln share one table set; rsqrt/sqrt
     sets avoided), row-side variant 2*inv via bias=ln(2)
  -> PE transpose + reshape-DMA + broadcast-DMA => inv_bc [128, 4096]
  -> PE Gram slab G = z_loc^T z (bf16, raw)
  -> DVE fold: G * inv_col        (one pass, PSUM->SBUF)
  -> ACT exp(scale_AP * x), scale = 2*inv_row per partition, fused row-sum
     accumulation (accum_out)
  -> pos via identity-mask on the fold output; final ln(rowsum - e^2)
  -> [128,1] partial per core; host sums 8x128 partials.
"""

import sys

sys.path.insert(0, "/opt/trn_rl_repo")

from contextlib import ExitStack

import ml_dtypes
import numpy as np

import concourse.bacc as bacc
import concourse.bass as bass
import concourse.mybir as mybir
import concourse.tile as tile
from concourse import bass_utils
from concourse.masks import make_identity
from concourse.vector_clock import ScopedClock
from concourse.tile_rust import add_dep_helper


def _cheap_drain_and_barrier(self, tick_clock, wait_clock):
    """Kernel tail: skip the second all-engine barrier of the stock
    TileContext epilogue (drain -> barrier -> sem clear). The engine
    streams end right after the clears, so NEFF completion already
    orders them; saves several us of fixed tail."""
    drain_inst = self.nc.sync.drain()
    wait_clock.add_sem_waits(
        drain_inst.ins, ScopedClock({None: tick_clock.global_clock})
    )
    self.nc.all_engine_barrier()
    popped = self.nc._tile_sem_poison_stack.pop()
    assert popped is self._sem_poison
    self.nc.clear_and_free_semaphores(list(self.sems.allocated().values()))

P = 128                  # partitions
D = 512                  # feature dim
N = 4096                 # 2B rows
KT = D // P              # 4 contraction tiles
NCHUNK = 4               # column chunks (1MB DMAs)
CH = N // NCHUNK         # 1024
IBC = CH // P            # 8 diag blocks per chunk
NB = N // P              # 32 diag blocks
ROWS = N // 8            # 512 rows per core
MT = ROWS // P           # 4 m-tiles per core
GW = 1024                # gram/psum group width (2 PSUM banks)
NQ = N // GW             # 4 groups per m-tile row
POSQ = 2                 # group holding the positive-pair columns
INV_T = 2.0              # 1 / temperature
E2 = float(np.exp(2.0))
LN2 = float(np.log(2.0))

F32 = mybir.dt.float32
BF16 = mybir.dt.bfloat16
FP8 = mybir.dt.float8e4
AF = mybir.ActivationFunctionType
ALU = mybir.AluOpType


def _bc_mid(ap: bass.AP, count: int) -> bass.AP:
    """Broadcast a [P, X] AP over an inserted middle dim: [P, count, X]."""
    return bass.AP(tensor=ap.tensor, offset=ap.offset,
                   ap=[ap.ap[0], [0, count], ap.ap[1]])


def _build_body(ctx: ExitStack, tc: tile.TileContext, zt_ap: bass.AP, out_ap: bass.AP):
    nc = tc.nc
    ztp = zt_ap.rearrange("(k p) n -> p k n", p=P)

    const = ctx.enter_context(tc.tile_pool(name="const", bufs=1))
    big = ctx.enter_context(tc.tile_pool(name="big", bufs=1))
    work = ctx.enter_context(tc.tile_pool(name="work", bufs=3))
    small = ctx.enter_context(tc.tile_pool(name="small", bufs=1))
    ps = ctx.enter_context(tc.tile_pool(name="ps", bufs=1, space="PSUM"))

    ident = const.tile([P, P], F32)
    make_identity(nc, ident)
    neg_e2 = const.tile([P, 1], F32)
    nc.vector.memset(neg_e2, -E2)
    ones = const.tile([P, P], BF16)
    nc.vector.memset(ones, 1.0)

    zb = big.tile([P, KT, N], FP8)        # 16KB/part
    inv_bc = big.tile([P, N], F32)        # 16KB/part
    inv2_pm = small.tile([P, MT], F32)    # 2/||z||, local rows, partition-major
    rs = small.tile([P, MT, NQ], F32)
    pos4raw = small.tile([P, MT], F32)
    pos4 = small.tile([P, MT], F32)
    rsum4 = small.tile([P, MT], F32)
    lnoff4 = small.tile([P, MT], F32)
    res4 = small.tile([P, MT], F32)
    outsb = small.tile([P, 1], F32)

    # ---- phase 1: per chunk, DMA -> square (ACT/DVE split) -> norms^2 via
    # ones-matmul (partition reduction, output broadcast across partitions)
    # -> absrsqrt straight into inv_bc. No transposes, no DRAM bounces. ----
    for c in range(NCHUNK):
        cs = slice(c * CH, (c + 1) * CH)
        nc.sync.dma_start(out=zb[:, :, cs], in_=ztp[:, :, cs])
        zsq = work.tile([P, KT, CH], BF16, tag="zsq", bufs=2, name=f"zsq{c}")
        # split the squaring across ACT / DVE / GPSIMD so the norms chain
        # clears as early as possible (phase-1 is elementwise-bound)
        if c == 0:
            nc.scalar.activation(out=zsq, in_=zb[:, :, cs], func=AF.Square)
        elif c == 1:
            nc.vector.tensor_mul(zsq, zb[:, :, cs], zb[:, :, cs])
        elif c == 2:
            nc.scalar.activation(out=zsq[:, 0:2, :], in_=zb[:, 0:2, cs],
                                 func=AF.Square)
            nc.vector.tensor_mul(zsq[:, 2:4, :], zb[:, 2:4, cs],
                                 zb[:, 2:4, cs])
        else:
            nc.gpsimd.tensor_mul(zsq[:, 0:2, :], zb[:, 0:2, cs],
                                 zb[:, 0:2, cs])
            nc.gpsimd.tensor_mul(zsq[:, 2:4, :], zb[:, 2:4, cs],
                                 zb[:, 2:4, cs])
        for half in range(2):
            s = c * CH + half * 512
            nsq_ps = ps.tile([P, 512], F32, tag="pre", bufs=2,
                             name=f"nsq{c}{half}")
            for k in range(KT):
                nc.tensor.matmul(
                    nsq_ps, lhsT=ones[:], rhs=zsq[:, k, half * 512 : (half + 1) * 512],
                    start=(k == 0), stop=(k == KT - 1),
                )
            nc.scalar.activation(out=inv_bc[:, s : s + 512], in_=nsq_ps,
                                 func=AF.Abs_reciprocal_sqrt)

    # ---- row-side scale 2/||z|| partition-major via tiny PE transposes of
    # the local 512 columns of inv_bc (values are partition-replicated) ----
    tps4 = ps.tile([P, MT, P], F32, tag="pre", bufs=2)
    for t in range(MT):
        nc.tensor.transpose(tps4[:, t, :], inv_bc[:, t * P : (t + 1) * P],
                            ident[:])
    nc.scalar.activation(out=inv2_pm, in_=tps4[:, :, 0], func=AF.Copy,
                         scale=2.0)

    # ---- phase 2: Gram slab + fold + exp row-sums + positives ----
    gi = 0
    for t in range(MT):
        ts_ = slice(t * P, (t + 1) * P)
        for q in range(NQ):
            gps = ps.tile([P, GW], F32, tag="gps", bufs=3, name=f"g{t}{q}")
            for k in range(KT):
                for j in range(GW // 512):
                    cols = slice(q * GW + j * 512, q * GW + (j + 1) * 512)
                    nc.tensor.matmul(
                        gps[:, j * 512 : (j + 1) * 512],
                        lhsT=zb[:, k, ts_], rhs=zb[:, k, cols],
                        start=(k == 0), stop=(k == KT - 1),
                    )
            gi += 1
            fold = work.tile([P, GW], F32, tag="fold", name=f"f{t}{q}")
            nc.vector.tensor_mul(fold, gps, inv_bc[:, q * GW : (q + 1) * GW])
            expo = work.tile([P, GW], BF16, tag="expo", bufs=2, name=f"e{t}{q}")
            nc.scalar.activation(
                out=expo, in_=fold, func=AF.Exp,
                scale=inv2_pm[:, t : t + 1],
                accum_out=rs[:, t, q : q + 1],
            )
            if q == POSQ:
                posscr = work.tile([P, P], F32, tag="posscr", name=f"p{t}")
                nc.vector.tensor_mul(posscr, fold[:, t * P : (t + 1) * P], ident)
                nc.vector.tensor_reduce(
                    out=pos4raw[:, t : t + 1], in_=posscr,
                    axis=mybir.AxisListType.X, op=ALU.add,
                )

    # ---- phase 3: per-row loss, partial reduction ----
    nc.vector.tensor_reduce(out=rsum4, in_=rs, axis=mybir.AxisListType.X,
                            op=ALU.add)
    nc.scalar.activation(out=lnoff4, in_=rsum4, func=AF.Ln, bias=neg_e2[:])
    nc.vector.tensor_mul(pos4, pos4raw, inv2_pm[:, 0:MT])
    nc.vector.tensor_sub(res4, lnoff4, pos4)
    nc.vector.tensor_reduce(out=outsb, in_=res4, axis=mybir.AxisListType.X,
                            op=ALU.add)
    nc.sync.dma_start(out=out_ap, in_=outsb)


_CACHE = {}


def _get_nc():
    if "nc" not in _CACHE:
        tile.TileContext._drain_and_barrier = _cheap_drain_and_barrier
        nc = bacc.Bacc("TRN2", target_bir_lowering=False, debug=False, num_devices=8)
        zt = nc.dram_tensor("zt", [D, N], FP8, kind="ExternalInput")
        out = nc.dram_tensor("out", [P, 1], F32, kind="ExternalOutput")
        with tile.TileContext(nc) as tc:
            with ExitStack() as ctx:
                _build_body(ctx, tc, zt.ap(), out.ap())
        nc.compile()
        _CACHE["nc"] = nc
    return _CACHE["nc"]


def _make_in_maps(proj_i: np.ndarray, proj_j: np.ndarray):
    z = np.concatenate(
        [np.asarray(proj_i, np.float32), np.asarray(proj_j, np.float32)], axis=0
    )
    zT = np.ascontiguousarray(z.T).astype(ml_dtypes.float8_e4m3fn)  # [D, N]
    return [
        {"zt": np.ascontiguousarray(np.roll(zT, -c * ROWS, axis=1))} for c in range(8)
    ]


def run(proj_i: np.ndarray, proj_j: np.ndarray, trace: bool = False):
    nc = _get_nc()
    in_maps = _make_in_maps(proj_i, proj_j)
    res = bass_utils.run_bass_kernel_spmd(
        nc, in_maps, core_ids=list(range(8)), trace=trace
    )
    loss = sum(float(r["out"].astype(np.float64).sum()) for r in res.results) / N
    return np.float32(loss), res


def kernel(proj_i: np.ndarray, proj_j: np.ndarray) -> np.ndarray:
    loss, _ = run(proj_i, proj_j)
    return np.asarray(loss, dtype=np.float32)
